# revision 1
# baseline (speedup 1.0000x reference)
"""Multi-head attention unit (proj + softmax attention + out-proj + bias + GELU)
for Trainium2, SPMD across 8 NeuronCores.

Sharding: core c = (batch b=c//2, query-half j=c%2). Each core computes all 16
heads for its 1024 query rows of batch b. k/v projections for the FULL 2048
keys of batch b are computed per-core from full inputs (duplicated compute,
no collectives - deterministic, removes all cross-core waits).

Attention inner loop (per head-pair p, query-block qb, key-tile kt):
  - two K=64 score matmuls (h0 via PE rows 0-63, h1 via rows 64-127) into
    ONE [128,1024] PSUM tile [s_h0 | s_h1]
  - ONE exp activation over [128,1024] (the scalar engine is the kernel's
    secondary bottleneck at ~270us total; 1024-wide tiles amortize its
    ~200ns/instruction overhead)
  - two AV matmuls, stationary v with a ones column appended ([128,65]) so
    output row 64 accumulates the softmax denominator for free
Two-phase kt split (kts 0-7 for all 16 (p,qb) units, then kts 8-15) with a
DRAM spill of the partial ctx+denominator, so the single PSUM accumulator
bank per head never serializes units against still-running v projection.
Normalization per (p,qb,head): DVE copy of the denominator row, fast
reciprocal, cast, gpsimd partition_broadcast (full partition-0-based tiles
only - sliced APs silently corrupt on HW), DVE multiply into bf16 ctxn.
Out-projection per qb right after its pairs finish (overlaps the other qb's
attention), bias via K=1 ones-row matmul, exact-erf GELU on ACT.
"""

import os

import numpy as np

B, S, D, NH = 4, 2048, 1024, 16
HD = D // NH          # 64
NCORES = 8
QLEN = S // 2         # 1024 query rows per core
NKT = S // 128        # 16 kpos tiles
NDC = D // 128        # 8 contraction chunks
NP = NH // 2          # 8 head pairs

_CACHED_NC = None


def _build():
    import concourse.bacc as bacc
    import concourse.mybir as mybir
    import concourse.tile as tile

    F32 = mybir.dt.float32
    CDT = mybir.dt.bfloat16
    ACT = mybir.ActivationFunctionType

    nc = bacc.Bacc("TRN2", target_bir_lowering=False, debug=False)

    qT_in = nc.dram_tensor("qT_in", [D, QLEN], CDT, kind="ExternalInput")
    kT_in = nc.dram_tensor("kT_in", [D, S], CDT, kind="ExternalInput")
    vT_in = nc.dram_tensor("vT_in", [D, S], CDT, kind="ExternalInput")
    WqT = nc.dram_tensor("WqT", [D, D], CDT, kind="ExternalInput")
    WkT = nc.dram_tensor("WkT", [D, D], CDT, kind="ExternalInput")
    WvT = nc.dram_tensor("WvT", [D, D], CDT, kind="ExternalInput")
    WoT = nc.dram_tensor("WoT", [D, D], CDT, kind="ExternalInput")
    b_o = nc.dram_tensor("b_o", [1, D], CDT, kind="ExternalInput")
    out = nc.dram_tensor("out", [QLEN, D], F32, kind="ExternalOutput")
    # phase-1 spill of partial ctx (rows 0-63) + denominator (row 64),
    # per unit u = qb*8 + pair, per head
    KDEBUG = os.environ.get("KDEBUG", "0") == "1"
    if KDEBUG:
        ctxp_d = nc.dram_tensor("ctxp_d", [16, 2, 65, 512], F32,
                                kind="ExternalOutput")
        ctxn_d = nc.dram_tensor("ctxn_d", [128, NP, QLEN], CDT,
                                kind="ExternalOutput")
        dent_d = nc.dram_tensor("dent_d", [16, 2, 65, 512], F32,
                                kind="ExternalOutput")
    else:
        ctxp_d = nc.dram_tensor("ctxp_d", [16, 2, 65, 512], F32)

    from contextlib import ExitStack
    with tile.TileContext(nc) as tc, ExitStack() as es:
        ep = es.enter_context
        cpool = ep(tc.tile_pool(name="consts", bufs=1))
        wpool = ep(tc.tile_pool(name="wt", bufs=2))
        xpool = ep(tc.tile_pool(name="xin", bufs=2))
        vinpool = ep(tc.tile_pool(name="vin", bufs=3))
        respool = ep(tc.tile_pool(name="res", bufs=1))
        epool = ep(tc.tile_pool(name="exp", bufs=16))
        npool = ep(tc.tile_pool(name="norm", bufs=1))
        spillp = ep(tc.tile_pool(name="spill", bufs=2))
        opool = ep(tc.tile_pool(name="osb", bufs=2))
        mmps = ep(tc.tile_pool(name="mm_ps", bufs=2, space="PSUM"))
        sps = ep(tc.tile_pool(name="s_ps", bufs=2, space="PSUM"))
        cps = ep(tc.tile_pool(name="ctx_ps", bufs=1, space="PSUM"))

        # ---- constants ----
        ones_f = cpool.tile([128, 128], F32, tag="ones_f")
        nc.gpsimd.memset(ones_f[:], 1.0)
        ones = cpool.tile([128, 128], CDT, tag="ones_r")
        nc.vector.tensor_copy(ones[:], ones_f[:])
        bo_sb = cpool.tile([1, D], CDT, tag="bo")
        nc.sync.dma_start(bo_sb[:], b_o[:])

        # ---- SBUF-resident activations ----
        qT_sb = respool.tile([128, NDC, QLEN], CDT, tag="qT_sb", name="qT_sb")
        kT_sb = respool.tile([128, NDC, S], CDT, tag="kT_sb", name="kT_sb")
        # va_sb[p, kt, h, c] = v_nat[kt*128+p, h*64+c]; c=64 -> 1.0 so the
        # AV matmul's 65th output row is the softmax denominator for free
        va_sb = respool.tile([128, NKT, NH, HD + 1], CDT, tag="va_sb",
                             name="va_sb")
        ctxn = respool.tile([128, NP, QLEN], CDT, tag="ctxn", name="ctxn")

        def proj_dt(w_sb, x_sb, x_sbres, dt_, xb):
            """One [128,512] projection psum group: out chunk dt_, col block xb."""
            ps = mmps.tile([128, 512], F32, name="pp", tag="mmp")
            for dc in range(NDC):
                nc.tensor.matmul(
                    ps[:],
                    w_sb[:, dc, dt_ * 128:(dt_ + 1) * 128],
                    x_sb[:, dc, :],
                    start=(dc == 0),
                    stop=(dc == NDC - 1),
                )
            nc.vector.tensor_copy(
                x_sbres[:, dt_, xb * 512:(xb + 1) * 512], ps[:]
            )

        def load_x(x_dram, xb):
            x_sb = xpool.tile([128, NDC, 512], CDT, tag="xin")
            nc.sync.dma_start(
                x_sb[:],
                x_dram[:, xb * 512:(xb + 1) * 512].rearrange(
                    "(dc p) s -> p dc s", p=128
                ),
            )
            return x_sb

        def v_proj_kt(wv_sb, kt):
            """Project v for one 128-kpos tile into va_sb[:, kt] (natural)."""
            vin = vinpool.tile([128, NDC, 128], CDT, tag="vin")
            nc.gpsimd.dma_start(
                vin[:],
                vT_in[:, kt * 128:(kt + 1) * 128].rearrange(
                    "(dc p) s -> p dc s", p=128
                ),
            )
            for dbl in range(2):
                ps = mmps.tile([128, 512], F32, name="pp", tag="mmp")
                for dc in range(NDC):
                    nc.tensor.matmul(
                        ps[:],
                        vin[:, dc, :],
                        wv_sb[:, dc, dbl * 512:(dbl + 1) * 512],
                        start=(dc == 0),
                        stop=(dc == NDC - 1),
                    )
                nc.vector.tensor_copy(
                    va_sb[:, kt, dbl * 8:(dbl + 1) * 8, 0:HD],
                    ps[:].rearrange("p (h c) -> p h c", c=HD),
                )
            nc.vector.tensor_copy(va_sb[:, kt, :, HD], ones[:, 0:16])

        def attn_scores_exp(p, qb, kt):
            """Scores for both heads of pair p into one [128,1024] psum tile
            (h0 cols 0-511 via PE rows 0-63, h1 cols 512-1023 via rows
            64-127, concurrent row-tiles), then one exp -> e tile."""
            s = sps.tile([128, 1024], F32, name="sp", tag="sp")
            for h in range(2):
                base = h * HD
                nc.tensor.matmul(
                    s[:, h * 512:(h + 1) * 512],
                    kT_sb[base:base + HD, p, kt * 128:(kt + 1) * 128],
                    qT_sb[base:base + HD, p, qb * 512:(qb + 1) * 512],
                )
            e = epool.tile([128, 1024], CDT, tag="e")
            nc.scalar.activation(e[:], s[:], ACT.Exp, scale=float(HD) ** -0.5)
            return e

        # ================= emission =================
        # 1. k projection, full 2048 keys (duplicated across the pair; no
        #    collectives). xb-major so early kt tiles complete first.
        with nc.named_scope("proj_k"):
            wk_sb = wpool.tile([128, NDC, D], CDT, tag="wt")
            nc.sync.dma_start(
                wk_sb[:], WkT[:].rearrange("(dc p) d -> p dc d", p=128)
            )
            for xb in range(4):
                x_sb = load_x(kT_in, xb)
                for dt_ in range(NDC):
                    proj_dt(wk_sb, x_sb, kT_sb, dt_, xb)

        # 2. q projection chunk dt0 only (unblocks pair-0 scores);
        #    dt1-7 are emitted inside the phase-1 rounds.
        with nc.named_scope("proj_q"):
            wq_sb = wpool.tile([128, NDC, D], CDT, tag="wt")
            nc.sync.dma_start(
                wq_sb[:], WqT[:].rearrange("(dc p) d -> p dc d", p=128)
            )
            q_x = [load_x(qT_in, xb) for xb in range(2)]
            for xb in range(2):
                proj_dt(wq_sb, q_x[xb], qT_sb, 0, xb)

        # 3. pre-split scores+exp for pair 0 (both qbs), kts 0-7: lets the
        #    scalar engine start while the v projection runs on the PE.
        pre_es = {}
        with nc.named_scope("attn"):
            for qb in range(2):
                for kt in range(8):
                    pre_es[(0, qb, kt)] = attn_scores_exp(0, qb, kt)

        # 4. v projection kts 0-7
        with nc.named_scope("proj_v"):
            wv_sb = wpool.tile([128, NDC, D], CDT, tag="wt")
            nc.sync.dma_start(
                wv_sb[:], WvT[:].rearrange("(dc p) d -> p dc d", p=128)
            )
            for kt in range(8):
                v_proj_kt(wv_sb, kt)

        # ---- phase 1: kts 0-7 for all 16 units; spill partials ----
        def unit(p, qb, ph, es_pre=None):
            u = qb * 8 + p
            cabs = [cps.tile([65, 512], F32, name=f"c{h}", tag=f"c{h}")
                    for h in range(2)]
            cxs = None
            if ph == 1:
                # prefetch the phase-1 partials while this unit's matmuls run
                cxs = []
                for h in range(2):
                    cx = spillp.tile([65, 512], F32, tag="cxs")
                    nc.gpsimd.dma_start(cx[:], ctxp_d[u, h])
                    cxs.append(cx)
            for kk in range(0, 8, 2):
                es = []
                for k2 in range(2):
                    kt = ph * 8 + kk + k2
                    if es_pre is not None:
                        es.append(es_pre[(p, qb, kt)])
                    else:
                        es.append(attn_scores_exp(p, qb, kt))
                for k2 in range(2):
                    kt = ph * 8 + kk + k2
                    for h in range(2):
                        nc.tensor.matmul(
                            cabs[h][:],
                            va_sb[:, kt, 2 * p + h, :],
                            es[k2][:, h * 512:(h + 1) * 512],
                            start=(kk + k2 == 0), stop=(kk + k2 == 7),
                        )
            if ph == 0:
                for h in range(2):
                    cx = spillp.tile([65, 512], F32, tag="cxs")
                    nc.vector.tensor_copy(cx[:], cabs[h][:])
                    nc.gpsimd.dma_start(ctxp_d[u, h], cx[:])
            else:
                for h in range(2):
                    ct = spillp.tile([65, 512], F32, tag="ctm")
                    nc.vector.tensor_add(ct[:], cabs[h][:], cxs[h][:])
                    if KDEBUG:
                        nc.gpsimd.dma_start(dent_d[u, h], ct[:])
                    # normalization, baseline-proven shapes: standalone
                    # [1,512]/[64,512] tiles (partition_broadcast requires
                    # partition-0-based full-tile APs on hardware)
                    den_sb = npool.tile([1, 512], F32, tag="den_sb")
                    nc.vector.tensor_copy(den_sb[:], ct[64:65, :])
                    scratch = npool.tile([1, 512], F32, tag="recip_s")
                    nc.vector.reciprocal_approx_fast(
                        out=scratch[:], in_=den_sb[:])
                    recip = npool.tile([1, 512], CDT, tag="recip")
                    nc.vector.tensor_copy(recip[:], scratch[:])
                    bc_sb = npool.tile([HD, 512], CDT, tag="bc")
                    nc.gpsimd.partition_broadcast(bc_sb[:], recip[:])
                    nc.vector.tensor_mul(
                        ctxn[h * 64:(h + 1) * 64, p,
                             qb * 512:(qb + 1) * 512],
                        ct[0:64, :],
                        bc_sb[:],
                    )
                if KDEBUG:
                    nc.gpsimd.dma_start(
                        ctxn_d[:, p, qb * 512:(qb + 1) * 512],
                        ctxn[:, p, qb * 512:(qb + 1) * 512])

        with nc.named_scope("attn"):
            # pair 0, both qbs (scores/exps pre-emitted above)
            unit(0, 0, 0, es_pre=pre_es)
            unit(0, 1, 0, es_pre=pre_es)
            # remaining pairs; interleave leftover projection work so it
            # lands in the PE slack of the scalar-bound attention phase.
            for p in range(1, NP):
                with nc.named_scope("proj_q"):
                    for xb in range(2):
                        proj_dt(wq_sb, q_x[xb], qT_sb, p, xb)
                with nc.named_scope("proj_v"):
                    v_proj_kt(wv_sb, 7 + p)  # kts 8-14
                unit(p, 0, 0)
                unit(p, 1, 0)
            with nc.named_scope("proj_v"):
                v_proj_kt(wv_sb, 15)

        # ---- phase 2: kts 8-15, merge, normalize; outproj per qb ----
        wo_sb = wpool.tile([128, NDC, D], CDT, tag="wt")
        nc.sync.dma_start(
            wo_sb[:], WoT[:].rearrange("(dc p) d -> p dc d", p=128)
        )

        def outproj_qt(qt):
            for dbl in range(2):
                ps = mmps.tile([128, 512], F32, name="pp", tag="mmp")
                for pair in range(NP):
                    nc.tensor.matmul(
                        ps[:],
                        ctxn[:, pair, qt * 128:(qt + 1) * 128],
                        wo_sb[:, pair, dbl * 512:(dbl + 1) * 512],
                        start=(pair == 0),
                        stop=False,
                    )
                nc.tensor.matmul(
                    ps[:],
                    ones[0:1, 0:128],
                    bo_sb[0:1, dbl * 512:(dbl + 1) * 512],
                    start=False,
                    stop=True,
                )
                o_sb = opool.tile([128, 512], F32, tag="osb")
                nc.scalar.activation(o_sb[:], ps[:], ACT.Gelu)
                nc.sync.dma_start(
                    out[qt * 128:(qt + 1) * 128, dbl * 512:(dbl + 1) * 512],
                    o_sb[:],
                )

        with nc.named_scope("attn"):
            for qb in range(2):
                for p in range(NP):
                    unit(p, qb, 1)
                with nc.named_scope("outproj"):
                    for qt in range(qb * 4, (qb + 1) * 4):
                        outproj_qt(qt)
    nc.compile()
    return nc


def _get_nc():
    global _CACHED_NC
    if _CACHED_NC is None:
        _CACHED_NC = _build()
    return _CACHED_NC


def _to_dt(a):
    import ml_dtypes
    return np.ascontiguousarray(a, dtype=ml_dtypes.bfloat16)


def kernel(value, key_t, query, mask, W_q, W_k, W_v, W_o, b_o):
    from concourse.bass_utils import run_bass_kernel_spmd

    nc = _get_nc()

    value = np.asarray(value, dtype=np.float32)
    key_t = np.asarray(key_t, dtype=np.float32)
    query = np.asarray(query, dtype=np.float32)
    WqT = _to_dt(np.asarray(W_q, np.float32).T)
    WkT = _to_dt(np.asarray(W_k, np.float32).T)
    WvT = _to_dt(np.asarray(W_v, np.float32).T)
    WoT = _to_dt(np.asarray(W_o, np.float32).T)
    bo = _to_dt(np.asarray(b_o, np.float32).reshape(1, D))

    in_maps = []
    for c in range(NCORES):
        b, j = divmod(c, 2)
        qT = _to_dt(query[b].T[:, j * QLEN:(j + 1) * QLEN])
        kT = _to_dt(key_t[b].T)
        vT = _to_dt(value[b].T)
        in_maps.append({
            "qT_in": qT, "kT_in": kT, "vT_in": vT,
            "WqT": WqT, "WkT": WkT, "WvT": WvT, "WoT": WoT, "b_o": bo,
        })

    res = run_bass_kernel_spmd(nc, in_maps, core_ids=list(range(NCORES)))

    out = np.empty((B, S, D), np.float32)
    for c in range(NCORES):
        b, j = divmod(c, 2)
        out[b, j * QLEN:(j + 1) * QLEN, :] = res.results[c]["out"]
    kernel.last_results = res
    return out

